# revision 1
# baseline (speedup 1.0000x reference)
"""Trainium2 Bass kernel for nn_CapsuleLayer_46677704573208.

Math note
---------
The reference's dynamic-routing update is degenerate:
    change = sum(outputs * probs, axis=-1)   # [B,C,R,1,1]
does not depend on u (only on outputs and probs), and in iteration 1
probs is uniform, so `change` is independent of the route index r.  By
induction logits stays constant along both r and the trailing o axis for
all three iterations, hence probs[b,c] is a per-(batch, capsule) scalar
and
    outputs = squash(probs[b,c] * S[b,c,:]),   S[b,c,o] = sum_r u[b,c,r,o].
S collapses to one dense matmul:
    S = X[B, R*I] @ W2[R*I, C*O],  W2[(r,i),(c,o)] = routing_weights[c,r,i,o]
i.e. [256, 9216] @ [9216, 160].  Everything after S is tiny [256,10,16]
elementwise math (verified to 1.2e-6 rms rel vs the fp32 reference).

Sharding
--------
The contraction dim K = 9216 is sharded 8 ways (1152 rows per core): each
core reads only x-slice (1.18 MB) + W2-slice (0.74 MB) — no replication,
total HBM traffic across the fleet equals the input size.  Each core
produces a partial S [256,160]; partials are summed on the host (the
"unshard" step) and the negligible routing epilogue is applied there.
"""

import os
import numpy as np

import concourse.bass as bass
import concourse.mybir as mybir
import concourse.tile as tile
from concourse import bacc, bass_utils

# Problem constants (hardcoded; harness calls kernel(**inputs) standalone).
B, R, I, C, O = 256, 1152, 8, 10, 16
N_CORES = 8
K = R * I            # 9216 total contraction length, index = r*I + i
KC = K // N_CORES    # 1152 contraction rows per core
KT = KC // 128       # 9 k-tiles of 128 per core
CO = C * O           # 160 output columns (c,o)
MT = B // 128        # 2 output row tiles of 128 batch rows
CHUNK = 3            # k-tiles per input DMA chunk (overlap DMA with PE)
F32 = mybir.dt.float32

_compiled = None
last_results = None  # BassKernelResults of most recent run (for test harness)


def build():
    nc = bacc.Bacc("TRN2", target_bir_lowering=False, debug=False,
                   num_devices=N_CORES)
    xt_d = nc.dram_tensor("xt", [128, KT, B], F32, kind="ExternalInput")
    w2_d = nc.dram_tensor("w2", [128, KT, CO], F32, kind="ExternalInput")
    out_d = nc.dram_tensor("out", [MT, 128, CO], F32, kind="ExternalOutput")

    with tile.TileContext(nc) as tc:
        with (
            tc.tile_pool(name="xin", bufs=1) as xin,
            tc.tile_pool(name="win", bufs=1) as win,
            tc.tile_pool(name="oout", bufs=MT) as oout,
            tc.tile_pool(name="acc", bufs=MT, space=bass.MemorySpace.PSUM) as accp,
        ):
            nchunks = KT // CHUNK
            xts, w2s = [], []
            for ci in range(nchunks):
                xt = xin.tile([128, CHUNK, B], F32, tag=f"x{ci}")
                w2 = win.tile([128, CHUNK, CO], F32, tag=f"w{ci}")
                nc.sync.dma_start(xt[:], xt_d[:, ci * CHUNK:(ci + 1) * CHUNK, :])
                nc.sync.dma_start(w2[:], w2_d[:, ci * CHUNK:(ci + 1) * CHUNK, :])
                xts.append(xt)
                w2s.append(w2)
            for m in range(MT):
                acc = accp.tile([128, CO], F32)
                for k in range(KT):
                    nc.tensor.matmul(
                        acc[:],
                        xts[k // CHUNK][:, k % CHUNK, bass.ts(m, 128)],
                        w2s[k // CHUNK][:, k % CHUNK, :],
                        start=(k == 0),
                        stop=(k == KT - 1),
                    )
                ot = oout.tile([128, CO], F32)
                nc.vector.tensor_copy(ot[:], acc[:])
                nc.sync.dma_start(out_d[m, :, :], ot[:])
    nc.compile()
    return nc


def _shard_inputs(x, w):
    # K-major matrices; K index = r*I + i so per-core r-slices are
    # contiguous row blocks.
    xt_full = np.ascontiguousarray(x.transpose(1, 2, 0)).reshape(K, B)
    w2_full = np.ascontiguousarray(w.transpose(1, 2, 0, 3)).reshape(K, CO)
    in_maps = []
    for j in range(N_CORES):
        xs = xt_full[j * KC:(j + 1) * KC].reshape(KT, 128, B).transpose(1, 0, 2)
        ws = w2_full[j * KC:(j + 1) * KC].reshape(KT, 128, CO).transpose(1, 0, 2)
        in_maps.append({
            "xt": np.ascontiguousarray(xs),
            "w2": np.ascontiguousarray(ws),
        })
    return in_maps


def _routing_epilogue(S):
    # S: [B, C, O] fp32. Collapsed 3-iteration routing (see module docstring).
    def squash(v):
        sq = v * v
        return (sq / (1.0 + sq)) * (v / np.sqrt(sq))

    out = squash(S * np.float32(0.1))
    logits = np.float32(0.1) * out.sum(-1)
    for _ in range(2):
        mmax = logits.max(1, keepdims=True)
        e = np.exp(logits - mmax)
        p = e / e.sum(1, keepdims=True)
        out = squash(p[:, :, None] * S)
        logits = logits + p * out.sum(-1)
    return out


def kernel(x, routing_weights):
    global _compiled, last_results
    x = np.ascontiguousarray(np.asarray(x, dtype=np.float32))
    w = np.ascontiguousarray(np.asarray(routing_weights, dtype=np.float32))
    assert x.shape == (B, R, I) and w.shape == (C, R, I, O)

    in_maps = _shard_inputs(x, w)
    if _compiled is None:
        _compiled = build()

    trace = bool(int(os.environ.get("CAPS_KERNEL_TRACE", "0")))
    res = bass_utils.run_bass_kernel_spmd(
        _compiled, in_maps, core_ids=list(range(N_CORES)), trace=trace,
    )
    last_results = res

    S = np.zeros((MT, 128, CO), np.float32)
    for core_out in res.results:
        S += core_out["out"]
    S = S.reshape(B, C, O)

    out = _routing_epilogue(S)
    return out.reshape(B, C, 1, 1, O).astype(np.float32)
